# revision 11
# baseline (speedup 1.0000x reference)
"""Trainium2 Bass kernel for BitwiseTasNetBlock (v2).

Data-parallel over batch: 8 cores x 2 batch items. All activations bf16
(rel err ~1.06e-2 vs 2e-2 budget). Per layer:

  B(i):  depthwise dilated conv as diagonal matmuls on PE (3 taps, psum
         accum); BN1 applied via ACT scale/bias on psum->sbuf with
         causal-pad boundary strip fixups; PReLU2 fused; bn_stats2 on
         DVE. First RA_CHUNKS chunk-iters are emitted BEFORE sync1 and
         drained raw (ACT Copy) to scratch so the PE works through the
         collective; their BN1+PReLU runs later from scratch.
  sync2: 4KB AllReduce of (sum,sumsq) -> BN2 folded into w2 (bf16).
  CA(i): fused conv2(i) + conv1(i+1), conv1 software-pipelined one chunk
         behind conv2 (PE executes in emission order; conv1(c) depends
         on the C-drain of chunk c). Last layer: residual add (DVE STT
         on prefetched x chunks) + DMA out instead of conv1.
  sync1: AllReduce -> BN1 scale/shift for the next B phase.

Y1/P2 share one persistent SBUF buffer per channel-tile; chunks run in
reverse time order so P2[c] overwrites Y1[c] in place.
"""
import sys

sys.path.insert(0, "/opt/trn_rl_repo")
import numpy as np

L, CB, D, KTAP = 4, 256, 512, 3
B, T = 16, 4096
EPS = 1e-5
NCORES = 8
BLOC = B // NCORES  # 2 batch items per core
CHUNK = 512
NCT = T // CHUNK  # 8 time chunks per batch item
NCHUNKS = BLOC * NCT  # 16 chunk-iterations per phase
PAD = 16
DOFF = PAD  # Y1 data offset inside ybuf
YCOLS = PAD + T
NGLOB = float(B * T)  # global elements per channel
RA_CHUNKS = 4  # dconv chunk-iters emitted + raw-drained before sync1

# Chunks are processed in REVERSE time order in every phase. That makes
# Y1[c] dead right after dconv(c), so P2[c] overwrites Y1[c]'s own slot.
REV = [(b, c) for b in range(BLOC) for c in range(NCT - 1, -1, -1)]

# packed per-channel vector indices
(V_B1, V_BD, V_G1, V_BE1, V_G2, V_BE2, V_WSA, V_WS12, V_WS2,
 V_WD0, V_WD1, V_WD2) = range(12)
NVEC = 12

_cache = {}


def _build(a1_vals, a2_vals):
    import concourse.bass as bass
    import concourse.tile as tile
    from concourse import bacc, mybir

    f32 = mybir.dt.float32
    bf16 = mybir.dt.bfloat16
    Alu = mybir.AluOpType
    Act = mybir.ActivationFunctionType

    nc = bacc.Bacc(None, target_bir_lowering=False, debug=False, num_devices=NCORES)

    xin_d = nc.dram_tensor("xin", [BLOC, CB, T], f32, kind="ExternalInput")
    w1t_d = nc.dram_tensor("w1t", [L, 128, 2, 4, 128], f32, kind="ExternalInput")
    w2t_d = nc.dram_tensor("w2t", [L, 128, 4, 2, 128], f32, kind="ExternalInput")
    vecs_d = nc.dram_tensor("vecs", [128, L, NVEC, 4], f32, kind="ExternalInput")
    b2_d = nc.dram_tensor("b2v", [128, L, 2], f32, kind="ExternalInput")
    eye_d = nc.dram_tensor("eye", [128, 128], f32, kind="ExternalInput")
    out_d = nc.dram_tensor("out", [BLOC, CB, T], f32, kind="ExternalOutput")

    with tile.TileContext(nc) as tc:
        with (
            tc.tile_pool(name="ybufp", bufs=1) as ybufp,
            tc.tile_pool(name="xbufp", bufs=1) as xbufp,
            tc.tile_pool(name="constp", bufs=1) as constp,
            tc.tile_pool(name="wtmp", bufs=2) as wtmpp,
            tc.tile_pool(name="wr", bufs=2) as wrp,
            tc.tile_pool(name="raw", bufs=4 * RA_CHUNKS) as rawp,
            tc.tile_pool(name="stage", bufs=8) as stagep,
            tc.tile_pool(name="ot", bufs=4) as otp,
            tc.tile_pool(name="stats", bufs=2) as statsp,
            tc.tile_pool(name="vec", bufs=10) as vecp,
            # Separate PSUM rings for (A,B) vs C: sharing one FIFO ring
            # couples slot waits across phases -> scheduler deadlock.
            tc.tile_pool(name="psab", bufs=6, space="PSUM") as psp,
            tc.tile_pool(name="psc", bufs=2, space="PSUM") as pscp,
            tc.tile_pool(name="dram", bufs=4, space="DRAM") as dramp,
        ):
            # persistent Y1/P2 buffers, one per channel-tile of D (bf16)
            ybuf = [
                ybufp.tile([128, BLOC, YCOLS], bf16, tag=f"ybuf{ct}", name=f"ybuf{ct}")
                for ct in range(4)
            ]
            # persistent conv1 input (bf16): [kt, b, T]
            xbuf = xbufp.tile([128, 2, BLOC, T], bf16, tag="xbuf", name="xbuf")

            # constants
            vecs_sb = constp.tile([128, L, NVEC, 4], f32)
            b2_sb = constp.tile([128, L, 2], f32)
            eye_sb = constp.tile([128, 128], f32)
            nc.sync.dma_start(vecs_sb[:], vecs_d[:])
            nc.sync.dma_start(b2_sb[:], b2_d[:])
            nc.sync.dma_start(eye_sb[:], eye_d[:])

            # startup alignment: dummy AllReduce issued first so the first
            # stats collective starts with cores already aligned.
            aln_in = dramp.tile([128, 1], f32, tag="alnin")
            aln_out = dramp.tile([128, 1], f32, tag="alnout")
            alnsb = constp.tile([128, 1], f32, tag="alnsb")
            nc.vector.memset(alnsb[:], 0.0)
            nc.scalar.dma_start(aln_in[:], alnsb[:])
            nc.gpsimd.collective_compute(
                "AllReduce", Alu.add,
                replica_groups=[list(range(NCORES))],
                ins=[aln_in[:].opt()], outs=[aln_out[:].opt()],
            )

            # zero the causal pads
            zt = constp.tile([128, PAD], bf16, tag="zt")
            nc.vector.memset(zt[:], 0.0)
            epsc = constp.tile([128, 1], f32, tag="epsc")
            nc.vector.memset(epsc[:], EPS)
            for ct in range(4):
                for b in range(BLOC):
                    nc.vector.tensor_copy(ybuf[ct][:, b, 0:PAD], zt[:])

            def load_w1(i):
                w1tmp = wtmpp.tile([128, 2, 4, 128], f32, tag="w1tmp")
                nc.sync.dma_start(w1tmp[:], w1t_d[i])
                w1r = wrp.tile([128, 2, 4, 128], bf16, tag="w1r")
                nc.vector.tensor_copy(w1r[:], w1tmp[:])
                return w1r

            def load_w2(i):
                w2tmp = wtmpp.tile([128, 4, 2, 128], f32, tag="w2tmp")
                nc.scalar.dma_start(w2tmp[:], w2t_d[i])
                return w2tmp

            def build_diag(i):
                # diagonal dconv weight blocks: eye * wd_tap (per-partition)
                diagr = wrp.tile([128, 3, 4, 128], bf16, tag="diagr")
                for j in range(3):
                    for ct in range(4):
                        nc.vector.tensor_scalar(
                            diagr[:, j, ct, :], eye_sb[:],
                            vecs_sb[:, i, V_WD0 + j, ct:ct + 1], None,
                            op0=Alu.mult,
                        )
                return diagr

            def stat_sync(i, st, g_idx, be_idx):
                """(sum, sumsq) from bn_stats triples + AllReduce.

                st: [128, 4(ct), NCHUNKS, 6]; each 6 = two (count, mean, M2)
                triples. Returns (s4, t4) tiles [128, 4].
                """
                st3 = st[:].rearrange("p ct ch (h s) -> p ct (ch h) s", s=3)
                means = st3[:, :, :, 1]
                m2s = st3[:, :, :, 2]
                csb = vecp.tile([128, 8], f32, tag="csb")
                sums_r = vecp.tile([128, 4], f32, tag="sums_r")
                nc.vector.tensor_reduce(
                    sums_r[:], means, axis=mybir.AxisListType.X, op=Alu.add
                )
                nc.vector.tensor_scalar(
                    csb[:, 0:4], sums_r[:], float(CHUNK // 2), None, op0=Alu.mult
                )
                msq = vecp.tile([128, 4, 2 * NCHUNKS], f32, tag="msq", bufs=2)
                nc.vector.tensor_mul(msq[:], means, means)
                nc.vector.scalar_tensor_tensor(
                    msq[:], msq[:], float(CHUNK // 2), m2s,
                    op0=Alu.mult, op1=Alu.add,
                )
                nc.vector.tensor_reduce(
                    csb[:, 4:8], msq[:], axis=mybir.AxisListType.X, op=Alu.add
                )
                cin = dramp.tile([128, 8], f32, tag="cin")
                cout = dramp.tile([128, 8], f32, tag="cout")
                nc.sync.dma_start(cin[:], csb[:])
                nc.gpsimd.collective_compute(
                    "AllReduce", Alu.add,
                    replica_groups=[list(range(NCORES))],
                    ins=[cin[:].opt()], outs=[cout[:].opt()],
                )
                gsb = vecp.tile([128, 8], f32, tag="gsb")
                nc.sync.dma_start(gsb[:], cout[:])

                mean4 = vecp.tile([128, 4], f32, tag="mean4")
                nc.vector.tensor_scalar(
                    mean4[:], gsb[:, 0:4], 1.0 / NGLOB, None, op0=Alu.mult
                )
                var4 = vecp.tile([128, 4], f32, tag="var4")
                nc.vector.tensor_scalar(
                    var4[:], gsb[:, 4:8], 1.0 / NGLOB, None, op0=Alu.mult
                )
                m24 = vecp.tile([128, 4], f32, tag="m24")
                nc.vector.tensor_mul(m24[:], mean4[:], mean4[:])
                nc.vector.tensor_sub(var4[:], var4[:], m24[:])
                std4 = vecp.tile([128, 4], f32, tag="std4")
                nc.scalar.activation(std4[:], var4[:], Act.Sqrt, bias=epsc[:], scale=1.0)
                rstd4 = vecp.tile([128, 4], f32, tag="rstd4")
                nc.vector.reciprocal(rstd4[:], std4[:])
                s4 = vecp.tile([128, 4], f32, tag="s4")
                nc.vector.tensor_mul(s4[:], rstd4[:], vecs_sb[:, i, g_idx, :])
                t4 = vecp.tile([128, 4], f32, tag="t4")
                nc.vector.tensor_mul(t4[:], mean4[:], s4[:])
                nc.vector.tensor_sub(t4[:], vecs_sb[:, i, be_idx, :], t4[:])
                return s4, t4

            def conv1_chunk(i, w1r, st1, b, c):
                """conv1 matmuls + PReLU drain + bn_stats for chunk (b,c)."""
                a1i = float(a1_vals[i])
                for mt in range(4):
                    ps = psp.tile([128, CHUNK], f32, tag="ps")
                    for kt in range(2):
                        nc.tensor.matmul(
                            ps[:],
                            w1r[:, kt, mt, :],
                            xbuf[:, kt, b, CHUNK * c:CHUNK * (c + 1)],
                            start=(kt == 0), stop=(kt == 1),
                        )
                    ysl = ybuf[mt][:, b, DOFF + CHUNK * c:DOFF + CHUNK * (c + 1)]
                    nc.scalar.activation(
                        ysl, ps[:], Act.Prelu,
                        bias=vecs_sb[:, i, V_B1, mt:mt + 1], scale=1.0, alpha=a1i,
                    )
                    nc.vector.bn_stats(st1[:, mt, b * NCT + c, :], ysl)

            def emit_dconv(i, diagr, dil, b, c, ct):
                ps = psp.tile([128, CHUNK], f32, tag="ps",
                              name=f"dps_{i}_{b}_{c}_{ct}")
                base = DOFF + CHUNK * c
                for j in range(KTAP):
                    off = base - (2 - j) * dil
                    nc.tensor.matmul(
                        ps[:],
                        diagr[:, j, ct, :],
                        ybuf[ct][:, b, off:off + CHUNK],
                        start=(j == 0), stop=(j == KTAP - 1),
                    )
                return ps

            # ================= layer 0 phase A =================
            w1r = load_w1(0)
            st1 = statsp.tile([128, 4, NCHUNKS, 6], f32, tag="st1", name="st1_0")

            def load_x_chunk(idx, b, c):
                for kt in range(2):
                    tmp = stagep.tile([128, CHUNK], f32, tag="xstage")
                    dma_eng = nc.sync if kt == 0 else nc.scalar
                    dma_eng.dma_start(
                        tmp[:],
                        xin_d[b, 128 * kt:128 * (kt + 1), CHUNK * c:CHUNK * (c + 1)],
                    )
                    xsl = xbuf[:, kt, b, CHUNK * c:CHUNK * (c + 1)]
                    if (idx + kt) % 2 == 0:
                        nc.vector.tensor_copy(xsl, tmp[:])
                    else:
                        nc.scalar.activation(xsl, tmp[:], Act.Copy)

            # software-pipelined: load 2 chunks ahead of conv1
            for pidx in range(2):
                load_x_chunk(pidx, *REV[pidx])
            for idx, (b, c) in enumerate(REV):
                if idx + 2 < len(REV):
                    load_x_chunk(idx + 2, *REV[idx + 2])
                conv1_chunk(0, w1r, st1, b, c)

            diagr = build_diag(0)

            for i in range(L):
                dil = 2 ** i
                a2i = float(a2_vals[i])

                # ---- dconv run-ahead: emit + raw-drain before sync1 ----
                raw_tiles = {}
                for (b, c) in REV[:RA_CHUNKS]:
                    for ct in range(4):
                        ps = emit_dconv(i, diagr, dil, b, c, ct)
                        raw = rawp.tile([128, CHUNK], bf16, tag="raw",
                                        name=f"raw_{i}_{b}_{c}_{ct}")
                        nc.scalar.activation(raw[:], ps[:], Act.Copy)
                        raw_tiles[(b, c, ct)] = raw

                # ---- sync1 ----
                s1, t1 = stat_sync(i, st1, V_G1, V_BE1)
                # bias2 variants [128, 4]: t1*wsum + bd
                b2a4 = vecp.tile([128, 4], f32, tag="b2a4")
                b2b4 = vecp.tile([128, 4], f32, tag="b2b4")
                b2c4 = vecp.tile([128, 4], f32, tag="b2c4")
                for dst, widx in ((b2a4, V_WSA), (b2b4, V_WS12), (b2c4, V_WS2)):
                    nc.vector.tensor_mul(dst[:], t1[:], vecs_sb[:, i, widx, :])
                    nc.vector.tensor_add(dst[:], dst[:], vecs_sb[:, i, V_BD, :])

                # preload next-layer w2 during B
                w2tmp = load_w2(i)

                # ---- phase B: dconv + BN1-apply + prelu2 + stats2 ----
                st2 = statsp.tile([128, 4, NCHUNKS, 6], f32, tag="st2",
                                  name=f"st2_{i}")
                for (b, c) in REV:
                    is_ra = (b, c, 0) in raw_tiles
                    for ct in range(4):
                        po = DOFF + CHUNK * c
                        p2sl = ybuf[ct][:, b, po:po + CHUNK]
                        if is_ra:
                            src = raw_tiles[(b, c, ct)][:]
                        else:
                            src = emit_dconv(i, diagr, dil, b, c, ct)[:]
                        nc.scalar.activation(
                            p2sl, src, Act.Prelu,
                            bias=b2a4[:, ct:ct + 1], scale=s1[:, ct:ct + 1],
                            alpha=a2i,
                        )
                        if c == 0:
                            # causal-pad boundary: first dil cols saw only
                            # tap2, next dil cols saw taps 1+2
                            nc.scalar.activation(
                                ybuf[ct][:, b, po:po + dil], src[:, 0:dil],
                                Act.Prelu,
                                bias=b2c4[:, ct:ct + 1], scale=s1[:, ct:ct + 1],
                                alpha=a2i,
                            )
                            nc.scalar.activation(
                                ybuf[ct][:, b, po + dil:po + 2 * dil],
                                src[:, dil:2 * dil], Act.Prelu,
                                bias=b2b4[:, ct:ct + 1], scale=s1[:, ct:ct + 1],
                                alpha=a2i,
                            )
                        nc.vector.bn_stats(st2[:, ct, b * NCT + c, :], p2sl)

                # ---- sync2 + weight folding ----
                s2, t2 = stat_sync(i, st2, V_G2, V_BE2)
                w2r = wrp.tile([128, 4, 2, 128], bf16, tag="w2r")
                rec4 = vecp.tile([128, 4], f32, tag="rec4")
                nc.vector.reciprocal(rec4[:], s2[:])
                r24 = vecp.tile([128, 4, 2], bf16, tag="r24")
                nc.vector.tensor_mul(r24[:, :, 0], t2[:], rec4[:])
                nc.vector.tensor_mul(r24[:, :, 1], t2[:], rec4[:])
                for kt in range(4):
                    nc.vector.tensor_scalar(
                        w2r[:, kt, :, :], w2tmp[:, kt, :, :], s2[:, kt:kt + 1],
                        None, op0=Alu.mult,
                    )
                # bias3 = W2' @ (t2/s2) + b2  (2 cols per kt, same value)
                psb = pscp.tile([128, 2, 2], f32, tag="psc", name=f"psb_{i}")
                for mt in range(2):
                    for kt in range(4):
                        nc.tensor.matmul(
                            psb[:, mt, :],
                            w2r[:, kt, mt, :],
                            r24[:, kt, :],
                            start=(kt == 0), stop=(kt == 3),
                        )
                bias3 = []
                for mt in range(2):
                    b3 = vecp.tile([128, 1], f32, tag="b3")
                    nc.scalar.activation(
                        b3[:], psb[:, mt, 0:1], Act.Identity,
                        bias=b2_sb[:, i, mt:mt + 1], scale=1.0,
                    )
                    bias3.append(b3)

                # preload next-layer conv1 weights + diag
                if i < L - 1:
                    w1r = load_w1(i + 1)
                    diagr = build_diag(i + 1)
                    st1 = statsp.tile([128, 4, NCHUNKS, 6], f32, tag="st1",
                                      name=f"st1_{i + 1}")

                # ---- phase CA: conv2(i) [+ conv1(i+1) | + residual out] ----
                # conv1(i+1) is software-pipelined one chunk behind conv2(i)
                # (PE executes in emission order; conv1(c) depends on the
                # C-drain of chunk c, so emitting it immediately would stall
                # the PE behind the ACT drain).
                last = i == L - 1

                def prefetch_rt(rts, pidx):
                    b, c = REV[pidx]
                    for mt in range(2):
                        rt = stagep.tile([128, CHUNK], f32, tag="rt")
                        eng = nc.sync if mt == 0 else nc.scalar
                        eng.dma_start(
                            rt[:],
                            xin_d[b, 128 * mt:128 * (mt + 1),
                                  CHUNK * c:CHUNK * (c + 1)],
                        )
                        rts[(pidx, mt)] = rt

                rts = {}
                if last:
                    for pidx in range(2):
                        prefetch_rt(rts, pidx)
                for idx, (b, c) in enumerate(REV):
                    if last and idx + 2 < len(REV):
                        prefetch_rt(rts, idx + 2)
                    po = DOFF + CHUNK * c
                    for mt in range(2):
                        ps = pscp.tile([128, CHUNK], f32, tag="psc")
                        for kt in range(4):
                            nc.tensor.matmul(
                                ps[:],
                                w2r[:, kt, mt, :],
                                ybuf[kt][:, b, po:po + CHUNK],
                                start=(kt == 0), stop=(kt == 3),
                            )
                        if not last:
                            nc.scalar.activation(
                                xbuf[:, mt, b, CHUNK * c:CHUNK * (c + 1)],
                                ps[:], Act.Identity,
                                bias=bias3[mt][:], scale=1.0,
                            )
                        else:
                            rt = rts.pop((idx, mt))
                            ot = otp.tile([128, CHUNK], f32, tag="ot")
                            nc.vector.scalar_tensor_tensor(
                                ot[:], ps[:], bias3[mt][:], rt[:],
                                op0=Alu.add, op1=Alu.add,
                            )
                            nc.sync.dma_start(
                                out_d[b, 128 * mt:128 * (mt + 1),
                                      CHUNK * c:CHUNK * (c + 1)],
                                ot[:],
                            )
                    if not last and idx > 0:
                        pb, pc = REV[idx - 1]
                        conv1_chunk(i + 1, w1r, st1, pb, pc)
                if not last:
                    pb, pc = REV[-1]
                    conv1_chunk(i + 1, w1r, st1, pb, pc)

    nc.compile()
    return nc


def _prep_inputs(x, w1, b1, a1, g1, be1, wd, bd, a2, g2, be2, w2, b2):
    """Host-side packing. All weights binarized via sign()."""
    w1b = np.sign(w1[..., 0]).astype(np.float32)  # [L, D, CB]
    wdb = np.sign(wd[..., 0, :]) if wd.ndim == 4 else np.sign(wd[:, :, 0, :])
    wdb = wdb.astype(np.float32)  # [L, D, K]
    w2b = np.sign(w2[..., 0]).astype(np.float32)  # [L, CB, D]

    w1t = np.empty((L, 128, 2, 4, 128), np.float32)
    w2t = np.empty((L, 128, 4, 2, 128), np.float32)
    for i in range(L):
        # lhsT[k, m] = w[m_global, k_global]
        for kt in range(2):
            for mt in range(4):
                blk = w1b[i, 128 * mt:128 * (mt + 1), 128 * kt:128 * (kt + 1)]
                w1t[i, :, kt, mt, :] = blk.T
        for kt in range(4):
            for mt in range(2):
                blk = w2b[i, 128 * mt:128 * (mt + 1), 128 * kt:128 * (kt + 1)]
                w2t[i, :, kt, mt, :] = blk.T

    wsa = wdb.sum(-1)  # [L, D]
    ws12 = wdb[:, :, 1] + wdb[:, :, 2]
    ws2 = wdb[:, :, 2]
    vec_list = [b1, bd, g1, be1, g2, be2, wsa, ws12, ws2,
                wdb[:, :, 0], wdb[:, :, 1], wdb[:, :, 2]]
    vecs = np.empty((128, L, NVEC, 4), np.float32)
    for v, arr in enumerate(vec_list):
        # arr [L, D] -> [128(p), L, ct]
        vecs[:, :, v, :] = arr.reshape(L, 4, 128).transpose(2, 0, 1)
    b2v = b2.reshape(L, 2, 128).transpose(2, 0, 1).astype(np.float32)  # [128, L, 2]
    eye = np.eye(128, dtype=np.float32)
    return w1t, w2t, vecs, b2v, eye


def kernel(**inputs):
    from concourse.bass_utils import run_bass_kernel_spmd

    inputs = {k: np.asarray(v, dtype=np.float32) for k, v in inputs.items()}
    x = inputs["x"]
    w1t, w2t, vecs, b2v, eye = _prep_inputs(**inputs)

    key = "nc"
    if key not in _cache:
        _cache[key] = _build(inputs["a1"], inputs["a2"])
    nc = _cache[key]

    in_maps = []
    for i in range(NCORES):
        in_maps.append({
            "xin": np.ascontiguousarray(x[BLOC * i:BLOC * (i + 1)]),
            "w1t": w1t, "w2t": w2t, "vecs": vecs, "b2v": b2v, "eye": eye,
        })
    import os
    trace = bool(int(os.environ.get("BASS_KERNEL_TRACE", "0")))
    res = run_bass_kernel_spmd(
        nc, in_maps, core_ids=list(range(NCORES)), trace=trace,
    )
    _cache["last_results"] = res
    out = np.empty((B, CB, T), np.float32)
    for i in range(NCORES):
        out[BLOC * i:BLOC * (i + 1)] = res.results[i]["out"]
    return out
